# revision 1
# baseline (speedup 1.0000x reference)
"""Trainium2 Bass kernel for NeuronToSpatialGrid.

reference: w[p,n] = exp(-|c_p - x_n|^2 / 0.02); w /= sum_n w + 1e-8;
           out[b,e,gx,gy] = sum_n w[p,n] * F[n,e],  p = gx*64+gy.

Strategy (8 cores = 4 batches x 2 grid-halves of 2048 points):
  stage 1: exponent[n,p] = 2c_p.x_n - |c_p|^2 via K=15 bf16 matmuls using
           3-term bf16 splits of both factors (~fp32 precision, 1 cyc/row).
           Two n-blocks run CONCURRENTLY in separate PE row bands
           (tile_position (0,0)/(32,0)) with pos packed at partitions 0-14
           and 32-46 and coords replicated in both bands.
           Then ACT Exp(in*50 + bias_n), bias_n = -50|x_n|^2 -> wt [n,p].
  stage 2: num[e,p] = sum_n F[n,e] * wt[n,p] via f32r matmuls accumulated
           in PSUM over 32 n-blocks; den via DVE accumulation of wt tiles
           into acc then a single ones-column matmul; normalize with DVE
           reciprocal broadcast through a K=1 matmul.
  Emission is globally software-pipelined: the stage-1 pack for window
  idx+2 is emitted right after window idx's e-matmuls (crossing j-tile
  boundaries) so the ACT engine — the roofline here — never waits on PE.
"""

import os
import numpy as np
import ml_dtypes

import concourse.bass as bass
import concourse.tile as tile
from concourse import bacc, mybir, bass_utils

BF16 = ml_dtypes.bfloat16
B, N, E, G = 4, 4096, 256, 64
P = G * G
HALF = P // 2          # grid points per core
N_CORES = 8
NB = N // 128          # 32 n-blocks
NW = NB // 2           # 16 packed windows (2 blocks each) per p-tile
PJ = HALF // 512       # 4 p-tiles per core
NWIN = PJ * NW         # 64 global windows
SIGMA2 = 2.0 * 0.1 ** 2
SCALE = 1.0 / SIGMA2   # 50.0

_CACHE = {}
LAST_EXEC_NS = None
LAST_RESULTS = None


def _split3(v):
    t1 = v.astype(BF16)
    r1 = v - t1.astype(np.float64)
    t2 = r1.astype(BF16)
    r2 = r1 - t2.astype(np.float64)
    t3 = r2.astype(BF16)
    return t1, t2, t3


def _build(reps=1):
    if reps in _CACHE:
        return _CACHE[reps]
    f32 = mybir.dt.float32
    f32r = mybir.dt.float32r
    bf16 = mybir.dt.bfloat16

    nc = bacc.Bacc("TRN2", target_bir_lowering=False, debug=False,
                   enable_asserts=False, num_devices=N_CORES)

    feat_d = nc.dram_tensor("feat", [N, E], f32r, kind="ExternalInput").ap()
    posp_d = nc.dram_tensor("pos_pack", [64, NW * 128], bf16,
                            kind="ExternalInput").ap()
    crdr_d = nc.dram_tensor("crd_rep", [64, HALF], bf16,
                            kind="ExternalInput").ap()
    bias_d = nc.dram_tensor("bias2d", [128, NB], f32, kind="ExternalInput").ap()
    onec_d = nc.dram_tensor("ones_col", [128, 1], f32r, kind="ExternalInput").ap()
    oner_d = nc.dram_tensor("ones_row", [1, 128], f32r, kind="ExternalInput").ap()
    out_d = nc.dram_tensor("out", [E, HALF], f32, kind="ExternalOutput").ap()

    with tile.TileContext(nc) as tc:
        from contextlib import ExitStack
        with ExitStack() as ctx:
            const = ctx.enter_context(tc.tile_pool(name="const", bufs=1))
            featp = ctx.enter_context(tc.tile_pool(name="feat", bufs=1))
            wtp = ctx.enter_context(tc.tile_pool(name="wt", bufs=10))
            accp = ctx.enter_context(tc.tile_pool(name="acc", bufs=2))
            recp = ctx.enter_context(tc.tile_pool(name="rec", bufs=2))
            outp = ctx.enter_context(tc.tile_pool(name="outsb", bufs=2))
            ps1 = ctx.enter_context(tc.tile_pool(name="ps1", bufs=1, space="PSUM"))
            pse = ctx.enter_context(tc.tile_pool(name="pse", bufs=1, space="PSUM"))
            psnb = ctx.enter_context(tc.tile_pool(name="psnb", bufs=1, space="PSUM"))

            posp_sb = const.tile([64, NW * 128], bf16)
            crdr_sb = const.tile([64, HALF], bf16)
            bias_sb = const.tile([128, NB], f32)
            onec_sb = const.tile([128, 1], f32r)
            oner_sb = const.tile([1, 128], f32r)
            # warm up the ACT Exp function table before the first real Exp
            warm = const.tile([1, 8], f32)
            warm2 = const.tile([1, 8], f32)
            nc.vector.memset(warm[:], 0.0)
            nc.scalar.activation(warm2[:], warm[:],
                                 mybir.ActivationFunctionType.Exp)
            # small startup tiles so the first two packs don't wait on the
            # full posp/crdr transfers
            posp0_sb = const.tile([64, 256], bf16)
            crdr0_sb = const.tile([64, 512], bf16)
            nc.sync.dma_start(posp0_sb[:], posp_d[:, 0:256])
            nc.sync.dma_start(crdr0_sb[:], crdr_d[:, 0:512])
            nc.sync.dma_start(bias_sb[:], bias_d[:])
            nc.sync.dma_start(posp_sb[:], posp_d[:])
            nc.sync.dma_start(crdr_sb[:], crdr_d[:])
            nc.sync.dma_start(onec_sb[:], onec_d[:])
            nc.sync.dma_start(oner_sb[:], oner_d[:])

            feat_sb = featp.tile([128, NB * E], f32r)
            for i in range(NB):
                nc.sync.dma_start(feat_sb[:, i * E:(i + 1) * E],
                                  feat_d[i * 128:(i + 1) * 128, :])

            pools = dict(wtp=wtp, accp=accp, recp=recp, outp=outp,
                         ps1=ps1, pse=pse, psnb=psnb,
                         feat_sb=feat_sb, posp_sb=posp_sb, crdr_sb=crdr_sb,
                         posp0_sb=posp0_sb, crdr0_sb=crdr0_sb,
                         bias_sb=bias_sb, onec_sb=onec_sb, oner_sb=oner_sb,
                         out_d=out_d)
            if reps == 1:
                _emit(nc, pools)
            else:
                with tc.For_i(0, reps, 1):
                    _emit(nc, pools)

    nc.compile()
    _CACHE[reps] = nc
    return nc


def _emit(nc, pools):
    f32 = mybir.dt.float32
    f32r = mybir.dt.float32r
    wtp, accp, recp, outp = (pools["wtp"], pools["accp"], pools["recp"],
                             pools["outp"])
    ps1, pse, psnb = pools["ps1"], pools["pse"], pools["psnb"]
    feat_sb, posp_sb, crdr_sb = (pools["feat_sb"], pools["posp_sb"],
                                 pools["crdr_sb"])
    posp0_sb, crdr0_sb = pools["posp0_sb"], pools["crdr0_sb"]
    bias_sb, onec_sb, oner_sb = (pools["bias_sb"], pools["onec_sb"],
                                 pools["oner_sb"])
    out_d = pools["out_d"]

    s1_store = {}

    def pack_band(idx, bnd):
        j, g = divmod(idx, NW)
        if idx < 2:
            psrc, poff = posp0_sb, g * 128
            csrc, coff = crdr0_sb, 0
        else:
            psrc, poff = posp_sb, g * 128
            csrc, coff = crdr_sb, j * 512
        r0 = 32 * bnd
        # both packs prefetched two windows ahead; ring depth 3/2 gives
        # extra WAR slack (total PSUM: 3+2 here + 2 pse + 1 nb = 8 banks)
        if bnd == 0:
            s1 = ps1.tile([128, 512], f32, name="s1a", bufs=3)
        else:
            s1 = ps1.tile([128, 512], f32, name="s1b", bufs=2)
        nc.tensor.matmul(s1[:],
                         psrc[r0:r0 + 15, poff:poff + 128],
                         csrc[r0:r0 + 15, coff:coff + 512],
                         start=True, stop=True, tile_position=(r0, 0))
        s1_store.setdefault(idx, [None, None])[bnd] = s1

    def emit_tail1(st):
        j, acc_d, acc_p, eo0, eo1 = st
        dn = psnb.tile([1, 512], f32, tag="nb")
        nc.tensor.matmul(dn[:], onec_sb[:], acc_d[:], start=True, stop=False)
        nc.tensor.matmul(dn[:], onec_sb[:], acc_p[:], start=False, stop=True)
        rec = recp.tile([1, 512], f32r)
        with nc.allow_low_precision(reason="f32r is bit-identical to f32"):
            nc.vector.reciprocal(rec[:], dn[:])
        return (j, eo0, eo1, rec)

    def emit_tail2(st):
        j, eo0, eo1, rec = st
        bc = psnb.tile([128, 512], f32, tag="nb")
        nc.tensor.matmul(bc[:], oner_sb[:], rec[:], start=True, stop=True)
        o0 = outp.tile([128, 512], f32)
        o1 = outp.tile([128, 512], f32)
        nc.vector.tensor_mul(o0[:], eo0[:], bc[:])
        nc.sync.dma_start(out_d[0:128, j * 512:(j + 1) * 512], o0[:])
        nc.vector.tensor_mul(o1[:], eo1[:], bc[:])
        nc.sync.dma_start(out_d[128:256, j * 512:(j + 1) * 512], o1[:])

    pack_band(0, 0)
    pack_band(0, 1)
    pack_band(1, 0)
    pack_band(1, 1)

    POOL_SET = frozenset((2, 5, 7, 10, 13, 15, 18, 21, 23, 26, 29, 31))
    pend1 = pend2 = None
    e0 = e1 = acc_d = acc_p = None
    for idx in range(NWIN):
        j, g = divmod(idx, NW)
        if g == 0:
            e0 = pse.tile([128, 512], f32)
            e1 = pse.tile([128, 512], f32)
            acc_d = accp.tile([128, 512], f32r)
            acc_p = accp.tile([128, 512], f32r)
        s1_pair = s1_store.pop(idx)
        for bnd in range(2):
            i = 2 * g + bnd
            s1 = s1_pair[bnd]
            wt = wtp.tile([128, 512], f32r)
            nc.scalar.activation(wt[:], s1[:],
                                 mybir.ActivationFunctionType.Exp,
                                 bias=bias_sb[:, i:i + 1], scale=SCALE)
            st, sp = (i == 0), (i == NB - 1)
            nc.tensor.matmul(e0[:], feat_sb[:, i * E:i * E + 128],
                             wt[:], start=st, stop=sp)
            nc.tensor.matmul(e1[:], feat_sb[:, i * E + 128:(i + 1) * E],
                             wt[:], start=st, stop=sp)
            if sp:
                # copy e0/e1 out of PSUM right after stop so the 1-deep
                # pse ring is free for next j's start=True matmuls
                eo0 = outp.tile([128, 512], f32)
                eo1 = outp.tile([128, 512], f32)
                nc.vector.tensor_copy(eo0[:], e0[:])
                nc.vector.tensor_copy(eo1[:], e1[:])
            # den accumulation split across DVE and Pool so neither
            # engine exceeds the ACT roofline per j-tile
            with nc.allow_low_precision(reason="f32r is bit-identical to f32"):
                if i in POOL_SET:
                    if i == 2:
                        nc.gpsimd.tensor_copy(acc_p[:], wt[:])
                    else:
                        nc.gpsimd.tensor_add(acc_p[:], acc_p[:], wt[:])
                else:
                    if i == 0:
                        nc.vector.tensor_copy(acc_d[:], wt[:])
                    else:
                        nc.vector.tensor_add(acc_d[:], acc_d[:], wt[:])
        # stage-1 packs for window idx+2 emitted after this window's
        # e-matmuls so ACT never waits on PE (depth-2 prefetch)
        if idx + 2 < NWIN:
            pack_band(idx + 2, 0)
            pack_band(idx + 2, 1)
        if g == NW - 1:
            pend1 = (j, acc_d, acc_p, eo0, eo1)
        elif g == 1 and pend1 is not None and pend1[0] == j - 1:
            pend2 = emit_tail1(pend1)
            pend1 = None
        elif g == 2 and pend2 is not None and pend2[0] == j - 1:
            emit_tail2(pend2)
            pend2 = None
    emit_tail2(emit_tail1(pend1))


def _host_prep(neuron_features, positions):
    """Per-core input maps. Core c: batch c//2, grid half c%2."""
    lin = np.linspace(0.0, 1.0, G).astype(np.float32)
    gx, gy = np.meshgrid(lin, lin, indexing="ij")
    coords = np.stack([gx.ravel(), gy.ravel()], axis=-1).astype(np.float64)

    crd_rep_halves = []
    for h in range(2):
        c = coords[h * HALF:(h + 1) * HALF]
        cx1, cx2, cx3 = _split3(2.0 * c[:, 0])
        cy1, cy2, cy3 = _split3(2.0 * c[:, 1])
        cn1, cn2, cn3 = _split3(c[:, 0] ** 2 + c[:, 1] ** 2)
        rows = [cx1, cx2, cx1, cx2, cx3, cx1,
                cy1, cy2, cy1, cy2, cy3, cy1,
                -cn1, -cn2, -cn3]
        crd15 = np.stack(rows, axis=0).astype(BF16)
        crd_rep = np.zeros((64, HALF), dtype=BF16)
        crd_rep[0:15] = crd15
        crd_rep[32:47] = crd15
        crd_rep_halves.append(crd_rep)

    pos_packs, biases = [], []
    for b in range(B):
        x = positions[b, :, 0].astype(np.float64)
        y = positions[b, :, 1].astype(np.float64)
        x1, x2, x3 = _split3(x)
        y1, y2, y3 = _split3(y)
        one = np.ones(N, dtype=BF16)
        rows15 = np.stack([x1, x1, x2, x2, x1, x3,
                           y1, y1, y2, y2, y1, y3,
                           one, one, one], axis=0).astype(BF16)
        pos_pack = np.zeros((64, NW * 128), dtype=BF16)
        for g in range(NW):
            pos_pack[0:15, g * 128:(g + 1) * 128] = \
                rows15[:, (2 * g) * 128:(2 * g + 1) * 128]
            pos_pack[32:47, g * 128:(g + 1) * 128] = \
                rows15[:, (2 * g + 1) * 128:(2 * g + 2) * 128]
        pos_packs.append(pos_pack)
        bias = (-SCALE * (x * x + y * y)).astype(np.float32)
        biases.append(np.ascontiguousarray(bias.reshape(NB, 128).T))

    ones_col = np.ones((128, 1), np.float32)
    ones_row = np.ones((1, 128), np.float32)
    in_maps = []
    for c in range(N_CORES):
        b, h = divmod(c, 2)
        in_maps.append({
            "feat": np.ascontiguousarray(neuron_features[b]),
            "pos_pack": pos_packs[b],
            "crd_rep": crd_rep_halves[h],
            "bias2d": biases[b],
            "ones_col": ones_col,
            "ones_row": ones_row,
        })
    return in_maps


def kernel(neuron_features, positions):
    global LAST_EXEC_NS, LAST_RESULTS
    nf = np.ascontiguousarray(np.asarray(neuron_features, dtype=np.float32))
    pos = np.ascontiguousarray(np.asarray(positions, dtype=np.float32))
    nc = _build()
    in_maps = _host_prep(nf, pos)
    trace = bool(int(os.environ.get("KERNEL_TRACE", "0")))
    res = bass_utils.run_bass_kernel_spmd(nc, in_maps,
                                          core_ids=list(range(N_CORES)),
                                          trace=trace)
    LAST_RESULTS = res
    LAST_EXEC_NS = getattr(res, "exec_time_ns", None)
    full = np.empty((B, E, P), np.float32)
    for c in range(N_CORES):
        b, h = divmod(c, 2)
        full[b, :, h * HALF:(h + 1) * HALF] = res.results[c]["out"]
    return full.reshape(B, E, G, G)

